# revision 18
# baseline (speedup 1.0000x reference)
"""AnchorGenerator on 8 TRN2 NeuronCores.

The reference output depends only on H=W=512 (feature_map values are unused):
for each (y, x, s, r) the anchor row is
    [max(16x+8-hw, 0), max(16y+8-hh, 0), min(16x+8+hw, 8192), min(16y+8+hh, 8192)]
with hw/hh the 3x3 half-width/height tables.

Sharding: 64 grid rows per core. Per core the flat (294912, 4) f32 output slab
is exactly a [128, 9216] SBUF tile in partition-major order, with
partition p = (y_rel, x_half) and free index f = x_rel*36 + (s*3+r)*4 + c.
The unclamped value decomposes as a low-rank product:
    v[p, f] = C[f] + Ygrid[p]*my[f] + X[p]*mx[f]
where the per-core row offset (1024*core) is folded into C's y-columns so
Ygrid = 16*(p//2) <= 1008 is exactly representable in bf16. C is split into
two bf16 addends (C1+C2 ~ C to 2^-17 rel), giving a K=4 bf16 matmul per
512-wide chunk with fp32 PSUM accumulation (PE cost scales with K*N). A fused
DVE tensor_scalar (max 0, min 8192) clamps PSUM->SBUF and contiguous DMA
stores write the slab. Chunks are small at the head (stores start early) and
tail (final DMA receipt short). Per-core HBM traffic is the 4.72 MB output
plus ~110 KB of tables.

hw/hh are computed with jnp on the device (mirroring the reference's op
sequence) so non-IEEE sqrt/divide rounding matches the reference bit-for-bit.
"""

import numpy as np
import ml_dtypes

H = 512
W = 512
N_CORES = 8
ROWS_PER_CORE = H // N_CORES  # 64
P = 128                       # partitions = (y_rel, x_half)
XW = W // 2                   # 256 x-positions per partition
SR = 9                        # scale x ratio combos
FREE = XW * SR * 4            # 9216 floats per partition
K = 4                         # bf16 matmul contraction: C1, C2, Ygrid, X
CHUNKS = (128, 384, 1024, 1024, 1024, 1024, 1024, 1024, 1024, 1024, 384, 128)
HEAD_CHUNKS = 3               # chunks covered by the fast SWDGE head load
SMALL_POOL = {0, 1, 10, 11}   # chunks using the small PSUM pool
MM_N = 512                    # matmul free-dim (one PSUM bank)
COORD_MAX = 8192.0            # W*16 == H*16
PACKED = P + FREE             # lhsT columns then rhs columns, one input

_cache = {}


def _bf16_split(v, n):
    """Split f64 vector v into n bf16 addends, most-significant first."""
    parts = []
    rem = v.copy()
    for _ in range(n):
        p = rem.astype(ml_dtypes.bfloat16)
        parts.append(p)
        rem = rem - p.astype(np.float64)
    return parts


def _half_sizes():
    """hw, hh as (3,3) f32, matching the reference's jnp ops on this backend."""
    import jax.numpy as jnp

    scales = jnp.asarray((0.5, 1.0, 2.0), dtype=jnp.float32)
    ratios = jnp.asarray((0.5, 1.0, 2.0), dtype=jnp.float32)
    sqrt_r = jnp.sqrt(ratios)
    aw = 16.0 * scales[:, None] * sqrt_r[None, :]
    ah = 16.0 * scales[:, None] / sqrt_r[None, :]
    hw = np.asarray(aw / 2, dtype=np.float32)
    hh = np.asarray(ah / 2, dtype=np.float32)
    return hw, hh


def _tables():
    """Per-core packed bf16 input (K, PACKED): lhsT columns then rhs columns."""
    hw, hh = _half_sizes()
    off = np.stack([-hw, -hh, hw, hh], axis=-1).reshape(36).astype(np.float64)
    isx = np.tile(np.array([1.0, 0.0, 1.0, 0.0]), SR)  # c parity: x-coords even
    x_rel = np.arange(XW, dtype=np.float64)
    base = 8.0 + 16.0 * x_rel[:, None] * isx[None, :]  # (XW, 36)
    mx = np.broadcast_to(isx, (XW, 36)).reshape(FREE)
    my = 1.0 - mx
    my_b = my.astype(ml_dtypes.bfloat16)
    mx_b = mx.astype(ml_dtypes.bfloat16)

    p = np.arange(P)
    Ygrid = (16.0 * (p // 2)).astype(ml_dtypes.bfloat16)   # exact
    X = (4096.0 * (p % 2)).astype(ml_dtypes.bfloat16)      # exact
    ones = np.ones(P, ml_dtypes.bfloat16)

    packed = np.zeros((N_CORES, K, PACKED), ml_dtypes.bfloat16)
    for c in range(N_CORES):
        # fold the per-core row offset into C's y-columns
        Cc = (base + off[None, :] + 1024.0 * c * (1.0 - isx)[None, :]).reshape(FREE)
        C1, C2 = _bf16_split(Cc, 2)
        packed[c, :, :P] = np.stack([ones, ones, Ygrid, X])        # lhsT
        packed[c, :, P:] = np.stack([C1, C2, my_b, mx_b])          # rhs
    return packed


def build_nc():
    import concourse.bacc as bacc
    import concourse.mybir as mybir
    import concourse.tile as tile

    nc = bacc.Bacc(None)
    tabs_d = nc.declare_dram_parameter("tabs", [K, PACKED], mybir.dt.bfloat16, isOutput=False)
    out_d = nc.declare_dram_parameter("out", [P, FREE], mybir.dt.float32, isOutput=True)

    # lhsT + first chunks in a small SWDGE load (fast completion); rest HWDGE
    head = P + sum(CHUNKS[:HEAD_CHUNKS])
    with tile.TileContext(nc) as tc:
        with (
            tc.tile_pool(name="const", bufs=1) as cpool,
            tc.tile_pool(name="osb", bufs=4) as osb,
            tc.tile_pool(name="psb", bufs=3, space="PSUM") as psb,
            tc.tile_pool(name="pss", bufs=1, space="PSUM") as pss,
        ):
            tabs = cpool.tile([K, PACKED], mybir.dt.bfloat16)
            nc.gpsimd.dma_start(tabs[:, :head], tabs_d[:, :head])
            nc.gpsimd.dma_start(tabs[:, head:], tabs_d[:, head:])
            lhsT = tabs[:, :P]
            lo = 0
            for ci, size in enumerate(CHUNKS):
                pool = pss if ci in SMALL_POOL else psb
                acc = pool.tile([P, size], mybir.dt.float32,
                                tag="accs" if ci in SMALL_POOL else "accb")
                for m0 in range(0, size, MM_N):
                    n = min(MM_N, size - m0)
                    nc.tensor.matmul(
                        acc[:, m0:m0 + n],
                        lhsT,
                        tabs[:, P + lo + m0: P + lo + m0 + n],
                    )
                o = osb.tile([P, size], mybir.dt.float32, tag="o")
                nc.vector.tensor_scalar(
                    o[:], acc[:], 0.0, COORD_MAX,
                    mybir.AluOpType.max, mybir.AluOpType.min,
                )
                eng = nc.sync if ci % 2 == 0 else nc.scalar
                eng.dma_start(out_d[:, lo:lo + size], o[:])
                lo += size
    nc.compile()
    return nc


def kernel(feature_map: np.ndarray) -> np.ndarray:
    from concourse.bass_utils import run_bass_kernel_spmd

    if "tables" not in _cache:
        _cache["tables"] = _tables()
    packed = _cache["tables"]
    if "nc" not in _cache:
        _cache["nc"] = build_nc()
    nc = _cache["nc"]

    in_maps = [{"tabs": packed[c]} for c in range(N_CORES)]
    res = run_bass_kernel_spmd(nc, in_maps, core_ids=list(range(N_CORES)))
    return np.concatenate(
        [res.results[c]["out"].reshape(-1, 4) for c in range(N_CORES)], axis=0
    )
